# revision 12
# baseline (speedup 1.0000x reference)
"""ConvGRU Trainium2 Bass kernel.

Math: ConvGRU cell with 3 gates (z, r, q), each gate = depthwise 3x3 conv
(SAME) followed by pointwise 1x1 conv, weights int8-fake-quantized
per-tensor.

Strategy:
  - Data-parallel over batch: 8 images -> 8 NeuronCores, one image each.
  - Per gate, the 9 depthwise taps are split between two engines to
    balance load:
      * folded taps (PE): p += (Wp . diag(Wd_t)) @ shift_t(hx) as shifted
        matmuls accumulated in PSUM (shifts are free-dim AP offsets into a
        zero-padded SBUF image, W 128->132, H 64->66);
      * DVE taps: d' += Wd_t[c] * shift_t(hx) via scalar_tensor_tensor
        (per-partition scalar), then one pointwise matmul Wp @ d' joins the
        same PSUM accumulation.
  - Weights are factored: int8 integer parts (exact in bf16) go into the
    matmuls; per-tensor scales and the combined bias (Wp@bd + bp) are
    applied by the ScalarEngine fused into sigmoid/tanh.
  - Per-core image is processed in 22 row-windows (3 padded rows = 396
    cols per matmul, PSUM-bank sized); DVE accumulators cover groups of 2
    windows to amortize per-op overhead.
"""

import sys

sys.path.insert(0, "/opt/trn_rl_repo")

import ml_dtypes
import numpy as np

HID, INP, C = 128, 320, 448
B, H, W = 8, 64, 128
Wp, Hp = 132, 66
PIX = Hp * Wp  # 8712
NPIX = H * W  # 8192
ROWS_PER_WIN = 3
NWIN = 22  # 21 windows x 3 rows + 1 window x 1 row
WIN_GROUP = 2

# channel chunks on partitions: [0:128)=h/rh, [128:256)=x0, [256:384)=x1, [384:448)=x2
CHUNKS = [(0, 128), (128, 256), (256, 384), (384, 448)]

# tap split per gate: F = folded on PE (center tap first: full-width matmul
# initializes every PSUM column of the group), U = DVE taps ((-1,0) first:
# it is full-width in every window so it can initialize d' without memset).
U_POOL = [(-1, 0), (-1, -1), (-1, 1)]
F_BASE = [(0, 0), (0, -1), (0, 1), (1, 0), (1, -1), (1, 1)]
FOLD_EXTRA = {"z": 0, "r": 0, "q": 0}


def _split(g):
    extra = FOLD_EXTRA[g]
    nu = len(U_POOL) - extra
    return F_BASE + U_POOL[nu:], U_POOL[:nu]


_CACHE = {}


def _win_geom(w):
    rows = ROWS_PER_WIN if w < NWIN - 1 else H - ROWS_PER_WIN * (NWIN - 1)
    q0 = (1 + ROWS_PER_WIN * w) * Wp
    return q0, rows, rows * Wp


def _groups():
    gs = []
    for w0 in range(0, NWIN, WIN_GROUP):
        wins = [_win_geom(w) for w in range(w0, min(w0 + WIN_GROUP, NWIN))]
        g0 = wins[0][0]
        ng = sum(n for _, _, n in wins)
        gs.append((w0, g0, ng, wins))
    return gs


def _build(loop_reps=None):
    """Build the SPMD kernel. loop_reps wraps the whole body in an on-device
    For_i loop (identical code size for any trip count) — used by test.py to
    measure device time as a wall-clock slope between two trip counts."""
    import contextlib

    import concourse.bacc as bacc
    import concourse.tile as tile
    from concourse import mybir

    f32 = mybir.dt.float32
    bf16 = mybir.dt.bfloat16
    AF = mybir.ActivationFunctionType
    ALU = mybir.AluOpType

    nc = bacc.Bacc("TRN2", target_bir_lowering=False, debug=False, num_devices=8)

    h32 = nc.dram_tensor("h32", [HID, NPIX], f32, kind="ExternalInput")
    x32 = nc.dram_tensor("x32", [INP, NPIX], f32, kind="ExternalInput")
    nF = {g: len(_split(g)[0]) for g in ("z", "r", "q")}
    wz = nc.dram_tensor("wz", [nF["z"], C, HID], bf16, kind="ExternalInput")
    wr = nc.dram_tensor("wr", [nF["r"], C, HID], bf16, kind="ExternalInput")
    wq = nc.dram_tensor("wq", [nF["q"], C, HID], bf16, kind="ExternalInput")
    wpw_d = nc.dram_tensor("wpw", [3, C, HID], bf16, kind="ExternalInput")
    dws_d = nc.dram_tensor("dws", [C, 16], f32, kind="ExternalInput")
    sbt_d = nc.dram_tensor("sbt", [HID, 6], f32, kind="ExternalInput")
    out_d = nc.dram_tensor("out", [HID, NPIX], f32, kind="ExternalOutput")

    with tile.TileContext(nc) as tc:
        with (
            tc.tile_pool(name="big", bufs=1) as big,
            tc.tile_pool(name="wp", bufs=1) as wpool,
            tc.tile_pool(name="stage", bufs=1) as stage,
            tc.tile_pool(name="win", bufs=3) as win,
            tc.tile_pool(name="dp", bufs=2) as dpool,
            tc.tile_pool(name="psum", bufs=2, space="PSUM") as psum,
            tc.For_i(0, loop_reps, 1) if loop_reps else contextlib.nullcontext(),
        ):
            hpad = big.tile([128, PIX], bf16)
            xc0 = big.tile([128, PIX], bf16)
            xc1 = big.tile([128, PIX], bf16)
            xc2 = big.tile([64, PIX], bf16)
            rhpad = big.tile([128, PIX], bf16)
            zpad = big.tile([128, PIX], bf16)

            for t_ in (hpad, xc0, xc1, xc2, rhpad):
                nc.vector.memset(t_[:], 0.0)

            sbt = wpool.tile([128, 6], f32)
            nc.sync.dma_start(out=sbt[:], in_=sbt_d[:])

            # folded-tap weight tiles [gate][fold-idx][chunk] + PW weights
            wd = {"z": wz, "r": wr, "q": wq}
            wt = {}
            wpw = {}
            dws = {}
            for g_i, g in enumerate(("z", "r", "q")):
                for t in range(nF[g]):
                    for ci, (c0, c1) in enumerate(CHUNKS):
                        wtile = wpool.tile(
                            [c1 - c0, 128], bf16, name=f"w_{g}_{t}_{ci}"
                        )
                        nc.sync.dma_start(out=wtile[:], in_=wd[g][t, c0:c1, :])
                        wt[(g, t, ci)] = wtile
                for ci, (c0, c1) in enumerate(CHUNKS):
                    ptile = wpool.tile([c1 - c0, 128], bf16, name=f"wp_{g}_{ci}")
                    nc.sync.dma_start(out=ptile[:], in_=wpw_d[g_i, c0:c1, :])
                    wpw[(g, ci)] = ptile
            for ci, (c0, c1) in enumerate(CHUNKS):
                dtile = wpool.tile([c1 - c0, 16], f32, name=f"dws_{ci}")
                nc.sync.dma_start(out=dtile[:], in_=dws_d[c0:c1, :])
                dws[ci] = dtile

            # load inputs + convert fp32 -> bf16 into padded layouts
            srcs = [hpad, xc0, xc1, xc2]
            for ci, (c0, c1) in enumerate(CHUNKS):
                kc = c1 - c0
                dst3 = srcs[ci].rearrange("p (r c) -> p r c", c=Wp)
                for half in range(2):
                    st = stage.tile([128, NPIX // 2], f32, tag="st")
                    src = h32 if ci == 0 else x32
                    off = 0 if ci == 0 else c0 - 128
                    nc.sync.dma_start(
                        out=st[:kc, :],
                        in_=src[
                            off : off + kc,
                            half * (NPIX // 2) : (half + 1) * (NPIX // 2),
                        ],
                    )
                    st3 = st.rearrange("p (r c) -> p r c", c=W)
                    nc.scalar.copy(
                        dst3[:kc, 1 + 32 * half : 33 + 32 * half, 1:129],
                        st3[:kc, :32, :],
                    )

            def dve_taps(gate, srcs_g, g0, ng, w0, tag):
                """Accumulate this gate's DVE taps into d' tiles for a
                window-group; returns the 4 per-chunk accumulator tiles.
                Uses tensor_scalar copies (4x mode) + tensor_tensor adds
                (2x) instead of scalar_tensor_tensor (1x only)."""
                _, U = _split(gate)
                dts = []
                for ci, (c0, c1) in enumerate(CHUNKS):
                    kc = c1 - c0
                    dt_ = dpool.tile(
                        [kc, WIN_GROUP * ROWS_PER_WIN * Wp], bf16,
                        tag=f"d{tag}{ci}", name=f"d{tag}{ci}_{w0}",
                    )
                    for ti, (dy, dx) in enumerate(U):
                        col = ti + 5 * ("zrq".index(gate))
                        o = g0 + dy * Wp + dx
                        s, e = max(o, 0), min(o + ng, PIX)
                        src = srcs_g[ci][:kc, s:e]
                        scal = dws[ci][:kc, col : col + 1]
                        if ti == 0:
                            # (-1,0): always full-width -> initializes dt_
                            nc.vector.tensor_scalar_mul(dt_[:kc, :ng], src, scal)
                        else:
                            tmp = dpool.tile(
                                [kc, WIN_GROUP * ROWS_PER_WIN * Wp], bf16,
                                tag=f"t{tag}{ci}", name=f"t{tag}{ci}_{w0}_{ti}",
                            )
                            cov = slice(s - o, s - o + (e - s))
                            nc.vector.tensor_scalar_mul(tmp[:kc, cov], src, scal)
                            nc.vector.tensor_add(
                                dt_[:kc, cov], dt_[:kc, cov], tmp[:kc, cov]
                            )
                    dts.append(dt_)
                return dts

            def gate_matmuls(gate, p, dts, srcs_g, q0, n, loc):
                """All PE matmuls for one gate in one window: folded taps
                then the pointwise pass over the DVE accumulator."""
                F, _ = _split(gate)
                nmm = (len(F) + 1) * 4
                i = 0
                for t, (dy, dx) in enumerate(F):
                    o = q0 + dy * Wp + dx
                    s, e = max(o, 0), min(o + n, PIX)
                    for ci, (c0, c1) in enumerate(CHUNKS):
                        kc = c1 - c0
                        nc.tensor.matmul(
                            p[:, s - o : s - o + (e - s)], wt[(gate, t, ci)][:],
                            srcs_g[ci][:kc, s:e],
                            start=(i == 0), stop=(i == nmm - 1),
                        )
                        i += 1
                for ci, (c0, c1) in enumerate(CHUNKS):
                    kc = c1 - c0
                    nc.tensor.matmul(
                        p[:], wpw[(gate, ci)][:], dts[ci][:kc, loc : loc + n],
                        start=False, stop=(i == nmm - 1),
                    )
                    i += 1

            zr_srcs = [hpad, xc0, xc1, xc2]
            q_srcs = [rhpad, xc0, xc1, xc2]

            # ---- phase 1: z and r gates; build rh = r*h and store z ----
            for w0, g0, ng, wins in _groups():
                dz = dve_taps("z", zr_srcs, g0, ng, w0, "a")
                dr = dve_taps("r", zr_srcs, g0, ng, w0, "b")
                for wi, (q0, rows, n) in enumerate(wins):
                    w = w0 + wi
                    loc = q0 - g0
                    pz = psum.tile([128, n], f32, tag="pz", name=f"pz{w}")
                    pr = psum.tile([128, n], f32, tag="pr", name=f"pr{w}")
                    gate_matmuls("z", pz, dz, zr_srcs, q0, n, loc)
                    gate_matmuls("r", pr, dr, zr_srcs, q0, n, loc)
                    rwin = win.tile([128, n], bf16, tag="rwin", name=f"rw{w}")
                    nc.scalar.activation(
                        rwin[:], pr[:], AF.Sigmoid,
                        bias=sbt[:, 3:4], scale=sbt[:, 2:3],
                    )
                    nc.scalar.activation(
                        zpad[:, q0 : q0 + n], pz[:], AF.Sigmoid,
                        bias=sbt[:, 1:2], scale=sbt[:, 0:1],
                    )
                    nc.vector.tensor_mul(
                        rhpad[:, q0 : q0 + n], rwin[:], hpad[:, q0 : q0 + n]
                    )

            # ---- phase 2: q gate + GRU mix ----
            out3 = out_d.rearrange("p (r c) -> p r c", c=W)
            for w0, g0, ng, wins in _groups():
                dq = dve_taps("q", q_srcs, g0, ng, w0, "a")
                for wi, (q0, rows, n) in enumerate(wins):
                    w = w0 + wi
                    loc = q0 - g0
                    pq = psum.tile([128, n], f32, tag="pq", name=f"pq{w}")
                    gate_matmuls("q", pq, dq, q_srcs, q0, n, loc)
                    qwin = win.tile([128, n], bf16, tag="qwin", name=f"qw{w}")
                    nc.scalar.activation(
                        qwin[:], pq[:], AF.Tanh, bias=sbt[:, 5:6], scale=sbt[:, 4:5]
                    )
                    dwin = win.tile([128, n], bf16, tag="dwin", name=f"dw{w}")
                    nc.vector.tensor_sub(dwin[:], qwin[:], hpad[:, q0 : q0 + n])
                    mwin = win.tile([128, n], bf16, tag="mwin", name=f"mw{w}")
                    nc.vector.tensor_mul(mwin[:], zpad[:, q0 : q0 + n], dwin[:])
                    owin = win.tile([128, n], f32, tag="owin", name=f"ow{w}")
                    nc.vector.tensor_add(owin[:], hpad[:, q0 : q0 + n], mwin[:])
                    o3 = owin.rearrange("p (r c) -> p r c", c=Wp)
                    y0 = ROWS_PER_WIN * w
                    nc.sync.dma_start(
                        out=out3[:, y0 : y0 + rows, :], in_=o3[:, :rows, 1:129]
                    )

    nc.compile()
    return nc


def _fq_int(w):
    w = np.asarray(w, np.float32)
    scale = (
        np.maximum(np.max(np.abs(w)), np.float32(1e-8)) / np.float32(127.0)
    ).astype(np.float32)
    q = np.clip(np.round(w / scale), -128, 127).astype(np.float32)
    return q, scale


def _prep_gate(g, wdg, bdg, wpg, bpg):
    qd, sd = _fq_int(wdg)  # [C,1,3,3]
    qp, sp = _fq_int(wpg)  # [HID,C,1,1]
    qp2 = qp[:, :, 0, 0]  # [HID, C]
    F, U = _split(g)
    lhsT = np.empty((len(F), C, HID), np.float32)
    for t, (dy, dx) in enumerate(F):
        ky, kx = dy + 1, dx + 1
        m = qp2 * qd[:, 0, ky, kx][None, :]  # [HID, C]
        lhsT[t] = m.T
    wpwT = qp2.T.copy()  # [C, HID]
    dwcol = np.zeros((C, 5), np.float32)
    for t, (dy, dx) in enumerate(U):
        dwcol[:, t] = qd[:, 0, dy + 1, dx + 1]
    scale = np.float32(sd) * np.float32(sp)
    bias = (
        np.float32(sp) * (qp2 @ np.asarray(bdg, np.float32))
        + np.asarray(bpg, np.float32)
    ).astype(np.float32)
    return lhsT.astype(ml_dtypes.bfloat16), wpwT, dwcol, scale, bias


def last_in_maps(inputs):
    h = np.asarray(inputs["h"], np.float32)
    x = np.asarray(inputs["x"], np.float32)

    wf = {}
    wpw = np.empty((3, C, HID), np.float32)
    dws = np.zeros((C, 16), np.float32)
    sbt = np.empty((HID, 6), np.float32)
    for gi, g in enumerate(("z", "r", "q")):
        lhsT, wpwT, dwcol, s, b = _prep_gate(
            g, inputs[f"wd{g}"], inputs[f"bd{g}"], inputs[f"wp{g}"],
            inputs[f"bp{g}"],
        )
        wf[g] = lhsT
        wpw[gi] = wpwT
        dws[:, 5 * gi : 5 * gi + 5] = dwcol
        sbt[:, 2 * gi] = s
        sbt[:, 2 * gi + 1] = b

    wpw_bf = wpw.astype(ml_dtypes.bfloat16)

    in_maps = []
    for i in range(B):
        in_maps.append(
            {
                "h32": np.ascontiguousarray(h[i].reshape(HID, NPIX)),
                "x32": np.ascontiguousarray(x[i].reshape(INP, NPIX)),
                "wz": wf["z"],
                "wr": wf["r"],
                "wq": wf["q"],
                "wpw": wpw_bf,
                "dws": dws,
                "sbt": sbt,
            }
        )
    return in_maps


def kernel(**inputs):
    from concourse.bass_utils import run_bass_kernel_spmd

    if "nc" not in _CACHE:
        _CACHE["nc"] = _build()
    nc = _CACHE["nc"]

    in_maps = last_in_maps(inputs)

    res = run_bass_kernel_spmd(nc, in_maps, list(range(B)))
    out = np.stack(
        [res.results[i]["out"].reshape(HID, H, W) for i in range(B)], axis=0
    )
    return out.astype(np.float32)
